# revision 1
# baseline (speedup 1.0000x reference)
"""Multi-head attention (B=2, S=2048, D=1024, H=16) on 8 Trainium2 NeuronCores.

Sharding: core c -> (batch b = c // 4, head-group hg = c % 4, 4 heads each).
Host sums the 4 partial output projections per batch and adds bias constants.

v6 changes vs v5:
- softmax 1/sum computed on the (otherwise idle) GPSIMD engine: ACT copies
  the replicated sums PSUM->SBUF, then GPSIMD does a magic-constant bit
  seed (int32: C - bits(x)) plus one Newton step (4 ops), fully deferred
  across the next hp. No Ln in the main loop -> single ACT table set,
  killing the 16 x 1.3us table reloads seen in v5.
- out-projection interleave moved to the next qt's hp1 (kts 2,6,10,14);
  Q projection interleave to hp0 (kts 2,9) for qt >= 1
- deferred DVE muls write both aoT halves at the next hp's kt14
- tail keeps the exp(-ln) path (2 table loads at the very end only)
"""
import sys

sys.path.insert(0, "/opt/trn_rl_repo")

import numpy as np
import ml_dtypes

N_CORES = 8
B, S, D = 2, 2048, 1024
H, DH = 16, 64
DLOC = D // 4  # 256 head dims per core
QT = 512
NQT = S // QT  # 4
KT = 128
NKT = S // KT  # 16
KC = D // 128  # 8 contraction chunks

SCH_A = float(np.float32(128.0 * np.log2(np.e) / 8.0))
SCH_B = float(np.float32(127.0 * 128.0 - 5.5))
DVE_BOTH = (3, 7, 11, 14)  # kts where DVE takes both heads
RECIP_MAGIC = 0x7EF477D5

_CACHE = {}


def _build():
    from concourse import bacc
    import concourse.mybir as mybir
    import concourse.tile as tile

    f32 = mybir.dt.float32
    bf16 = mybir.dt.bfloat16
    i16 = mybir.dt.int16
    i32 = mybir.dt.int32
    AF = mybir.ActivationFunctionType
    ALU = mybir.AluOpType

    nc = bacc.Bacc("TRN2", target_bir_lowering=False, debug=False,
                   num_devices=N_CORES)
    xq = nc.declare_dram_parameter("xq", [NQT, 128, KC, QT], bf16,
                                   isOutput=False)
    xk = nc.declare_dram_parameter("xk", [NQT, 128, KC, QT], bf16,
                                   isOutput=False)
    xv = nc.declare_dram_parameter("xv", [NQT, 128, KC, QT], bf16,
                                   isOutput=False)
    wq = nc.declare_dram_parameter("wq", [128, KC, DLOC], bf16, isOutput=False)
    wk = nc.declare_dram_parameter("wk", [128, KC, DLOC], bf16, isOutput=False)
    wv = nc.declare_dram_parameter("wv", [128, KC, DLOC], bf16, isOutput=False)
    wo = nc.declare_dram_parameter("wo", [128, 2, D], bf16, isOutput=False)
    bq = nc.declare_dram_parameter("bq", [128, 2], f32, isOutput=False)
    bk = nc.declare_dram_parameter("bk", [128, 2], f32, isOutput=False)
    yT = nc.declare_dram_parameter("yT", [8, 128, NQT, QT], bf16,
                                   isOutput=True)

    with tile.TileContext(nc) as tc:
        with (
            tc.tile_pool(name="keep", bufs=1) as keep,
            tc.tile_pool(name="big", bufs=1) as big,
            tc.tile_pool(name="xpool", bufs=4) as xpool,
            tc.tile_pool(name="esb", bufs=8) as esbp,
            tc.tile_pool(name="nrm", bufs=2) as nrmp,
            tc.tile_pool(name="ysb", bufs=2) as ysbp,
            tc.tile_pool(name="psS", bufs=4, space="PSUM") as psS,
            tc.tile_pool(name="psAV", bufs=2, space="PSUM") as psAV,
        ):
            # --- resident weights / constants ---
            wq_t = keep.tile([128, KC, DLOC], bf16, tag="wq")
            wk_t = keep.tile([128, KC, DLOC], bf16, tag="wk")
            wv_t = keep.tile([128, KC, DLOC], bf16, tag="wv")
            wo_t = keep.tile([128, 2, D], bf16, tag="wo")
            bq_t = keep.tile([128, 2], f32, tag="bq")
            bk_t = keep.tile([128, 2], f32, tag="bk")

            # persistent activations
            qh = big.tile([128, 2, S], bf16)   # [head-dim pair, hp, q]
            kh = big.tile([128, 2, S], bf16)
            vsb = big.tile([128, NKT, 4, 128], bf16)  # [kpos, kt, head, d|1s]
            aoT = big.tile([128, 2, S], bf16)  # attn out^T [dlocal, q]
            nc.vector.memset(vsb[:, :, :, 64:128], 1.0)

            # --- DMAs up-front in arrival order (wo late: needed at qt1) ---
            nc.sync.dma_start(out=wk_t, in_=wk[:, :, :])
            nc.sync.dma_start(out=bk_t, in_=bk[:, :])
            xtiles = {}

            def declare_x(kind, nt, chunks, axis="q"):
                x_d = {"q": xq, "k": xk, "v": xv}[kind]
                xc = xpool.tile([128, KC, QT], bf16, tag="xc")
                n = (KC if axis == "c" else QT) // chunks
                for ch in range(chunks):
                    sl = slice(ch * n, (ch + 1) * n)
                    if axis == "c":
                        nc.sync.dma_start(out=xc[:, sl, :],
                                          in_=x_d[nt, :, sl, :])
                    else:
                        nc.sync.dma_start(out=xc[:, :, sl],
                                          in_=x_d[nt, :, :, sl])
                xtiles[(kind, nt)] = xc

            declare_x("k", 0, 2, axis="c")
            nc.sync.dma_start(out=wq_t, in_=wq[:, :, :])
            nc.sync.dma_start(out=bq_t, in_=bq[:, :])
            declare_x("q", 0, 2, axis="c")
            nc.sync.dma_start(out=wv_t, in_=wv[:, :, :])
            declare_x("v", 0, 4)
            declare_x("k", 1, 2)
            declare_x("v", 1, 4)
            declare_x("k", 2, 2)
            declare_x("v", 2, 4)
            declare_x("k", 3, 2)
            declare_x("v", 3, 4)
            nc.sync.dma_start(out=wo_t, in_=wo[:, :, :])
            for nt in range(1, NQT):
                declare_x("q", nt, 2)

            def proj_qk_part(state, half):
                kind, nt, mt, o_t, w_t, b_t, ps = state
                xc = xtiles[(kind, nt)]
                for c in range(half * 4, half * 4 + 4):
                    nc.tensor.matmul(
                        ps,
                        w_t[:, c, mt * 128:(mt + 1) * 128],
                        xc[:, c, :],
                        start=(c == 0), stop=(c == KC - 1),
                        skip_group_check=True)
                if half == 1:
                    nc.scalar.activation(
                        o_t[:, mt, nt * QT:(nt + 1) * QT], ps,
                        AF.Identity, bias=b_t[:, mt:mt + 1])

            def proj_qk_mt(kind, nt, mt, o_t, w_t, b_t):
                ps = psS.tile([128, QT], f32, tag="sc")
                state = (kind, nt, mt, o_t, w_t, b_t, ps)
                proj_qk_part(state, 0)
                proj_qk_part(state, 1)

            def proj_v_st(st):
                # one 128-row chunk of sequence -> vsb[:, st, :, 0:64]
                nt, stl = st // 4, st % 4
                xc = xtiles[("v", nt)]
                ps = psS.tile([128, QT], f32, tag="sc")
                psv = ps[:, 0:DLOC]
                for c in range(KC):
                    nc.tensor.matmul(
                        psv,
                        xc[:, c, stl * 128:(stl + 1) * 128],
                        wv_t[:, c, :],
                        start=(c == 0), stop=(c == KC - 1),
                        skip_group_check=True)
                nc.scalar.copy(
                    vsb[:, st, :, 0:64],
                    psv.rearrange("p (h d) -> p h d", h=4))

            def out_proj_pair(qt, pt):
                # output rows [2pt*128, (2pt+2)*128) for q tile qt
                qsl = slice(qt * QT, (qt + 1) * QT)
                ysb = ysbp.tile([128, 2, QT], bf16, tag="y")
                for j in range(2):
                    py = psS.tile([128, QT], f32, tag="sc")
                    mt = 2 * pt + j
                    for c in range(2):
                        nc.tensor.matmul(
                            py,
                            wo_t[:, c, mt * 128:(mt + 1) * 128],
                            aoT[:, c, qsl],
                            start=(c == 0), stop=(c == 1),
                            skip_group_check=True)
                    if j == 0:
                        nc.scalar.copy(ysb[:, j, :], py)
                    else:
                        nc.vector.tensor_copy(ysb[:, j, :], py)
                    nc.sync.dma_start(out=yT[mt, :, qt, :],
                                      in_=ysb[:, j, :])

            def norm_muls(av, rcp, hp, qsl):
                nc.vector.tensor_mul(
                    aoT[0:64, hp, qsl], av[0:64, 0, :], rcp[:, 0, :])
                nc.vector.tensor_mul(
                    aoT[64:128, hp, qsl], av[0:64, 1, :], rcp[:, 1, :])

            # --- lead-in ---
            proj_qk_mt("k", 0, 0, kh, wk_t, bk_t)
            proj_qk_mt("k", 0, 1, kh, wk_t, bk_t)
            proj_qk_mt("q", 0, 0, qh, wq_t, bq_t)
            proj_qk_mt("q", 0, 1, qh, wq_t, bq_t)
            for st in range(4):
                proj_v_st(st)

            pending = None  # (av, hp, qsl) awaiting normalize
            ctx = {}

            # --- attention loop ---
            for qt in range(NQT):
                qsl = slice(qt * QT, (qt + 1) * QT)
                for hp in range(2):
                    av = psAV.tile([128, 2, QT], f32, tag="av")
                    for kt in range(NKT):
                        # deferred normalize chain for the previous (qt, hp)
                        if pending is not None:
                            if kt == 0:
                                sums = nrmp.tile([64, 2, QT], f32, tag="sums")
                                nc.scalar.copy(sums,
                                               pending[0][64:128, :, :])
                                ctx["sums"] = sums
                            elif kt == 1:
                                y0 = nrmp.tile([64, 2, QT], f32, tag="y0")
                                nc.gpsimd.tensor_scalar(
                                    y0[:].bitcast(i32),
                                    ctx["sums"][:].bitcast(i32),
                                    -1, RECIP_MAGIC, ALU.mult, ALU.add)
                                ctx["y0"] = y0
                            elif kt == 2:
                                m = nrmp.tile([64, 2, QT], f32, tag="m")
                                nc.gpsimd.tensor_mul(
                                    m, ctx["sums"], ctx["y0"])
                                ctx["m"] = m
                            elif kt == 3:
                                sn = nrmp.tile([64, 2, QT], f32, tag="s")
                                nc.gpsimd.tensor_scalar(
                                    sn, ctx["m"], -1.0, 2.0,
                                    ALU.mult, ALU.add)
                                ctx["s"] = sn
                            elif kt == 4:
                                rcp = nrmp.tile([64, 2, QT], f32, tag="rcp")
                                nc.gpsimd.tensor_mul(
                                    rcp, ctx["s"], ctx["y0"])
                                ctx["rcp"] = rcp
                            elif kt == 14:
                                norm_muls(pending[0], ctx["rcp"],
                                          pending[1], pending[2])
                                pending = None
                        # interleaved just-in-time work
                        if qt == 0 and hp == 0:
                            if kt < 6:  # K proj nt 1..3
                                proj_qk_mt("k", 1 + kt // 2, kt % 2,
                                           kh, wk_t, bk_t)
                            if 2 <= kt < 14:  # V proj st 4..15
                                proj_v_st(kt + 2)
                        if hp == 1 and qt == 0 and kt in (2, 9):
                            proj_qk_mt("q", 1, 0 if kt == 2 else 1,
                                       qh, wq_t, bq_t)
                        if hp == 0 and 0 < qt < NQT - 1 and kt in (6, 12):
                            mt = 0 if kt == 6 else 1
                            ps = psS.tile([128, QT], f32, tag="sc")
                            ctx["qs"] = ("q", qt + 1, mt, qh, wq_t, bq_t, ps)
                            proj_qk_part(ctx["qs"], 0)
                        if hp == 0 and 0 < qt < NQT - 1 and kt in (7, 13):
                            proj_qk_part(ctx["qs"], 1)
                        if hp == 1 and qt > 0 and kt in (2, 6, 10, 14):
                            out_proj_pair(qt - 1, (kt - 2) // 4)

                        scA = psS.tile([128, QT], f32, tag="sc")
                        scB = psS.tile([128, QT], f32, tag="sc")
                        ksl = slice(kt * 128, (kt + 1) * 128)
                        nc.tensor.matmul(
                            scA, kh[0:64, hp, ksl],
                            qh[0:64, hp, qsl], start=True, stop=True)
                        nc.tensor.matmul(
                            scB, kh[64:128, hp, ksl],
                            qh[64:128, hp, qsl], start=True, stop=True)
                        esb = esbp.tile([128, 2, QT], bf16, tag="e")
                        # one head's exp on ACT, the other on DVE, swapping
                        # each kt so every softmax row is a 50/50 mix
                        dve_j = kt % 2
                        for j, scj in ((0, scA), (1, scB)):
                            if j == dve_j or kt in DVE_BOTH:
                                nc.vector.tensor_scalar(
                                    esb[:, j, :].bitcast(i16), scj,
                                    SCH_A, SCH_B, ALU.mult, ALU.add)
                            else:
                                nc.scalar.activation(
                                    esb[:, j, :], scj, AF.Exp, scale=0.125)
                        nc.tensor.matmul(
                            av[:, 0, :], vsb[:, kt, 2 * hp, :], esb[:, 0, :],
                            start=(kt == 0), stop=(kt == NKT - 1),
                            skip_group_check=True)
                        nc.tensor.matmul(
                            av[:, 1, :], vsb[:, kt, 2 * hp + 1, :],
                            esb[:, 1, :],
                            start=(kt == 0), stop=(kt == NKT - 1),
                            skip_group_check=True)

                    pending = (av, hp, qsl)

            # tail: final normalize via exp(-ln); pairs 0,1 start their
            # hp0-side (c=0) matmuls under the normalize latency
            qsl3 = slice((NQT - 1) * QT, NQT * QT)
            early = []
            for pt in range(2):
                for j in range(2):
                    py = psS.tile([128, QT], f32, tag="sc")
                    nc.tensor.matmul(
                        py,
                        wo_t[:, 0, (2 * pt + j) * 128:(2 * pt + j + 1) * 128],
                        aoT[:, 0, qsl3],
                        start=True, stop=False, skip_group_check=True)
                    early.append(py)
            lnt = nrmp.tile([64, 2, QT], f32, tag="m")
            nc.scalar.activation(lnt, pending[0][64:128, :, :], AF.Ln)
            rcp = nrmp.tile([64, 2, QT], f32, tag="rcp")
            nc.scalar.activation(rcp, lnt, AF.Exp, scale=-1.0)
            norm_muls(pending[0], rcp, pending[1], pending[2])
            for pt in range(2):
                ysb = ysbp.tile([128, 2, QT], bf16, tag="y")
                for j in range(2):
                    py = early[2 * pt + j]
                    nc.tensor.matmul(
                        py,
                        wo_t[:, 1, (2 * pt + j) * 128:(2 * pt + j + 1) * 128],
                        aoT[:, 1, qsl3],
                        start=False, stop=True, skip_group_check=True)
                    nc.vector.tensor_copy(ysb[:, j, :], py)
                    nc.sync.dma_start(out=yT[2 * pt + j, :, NQT - 1, :],
                                      in_=ysb[:, j, :])
            for pt in range(2, 4):
                out_proj_pair(NQT - 1, pt)
    nc.compile()
    return nc


def _get_nc():
    if "nc" not in _CACHE:
        _CACHE["nc"] = _build()
    return _CACHE["nc"]


def kernel(q, k, v, w_q, b_q, w_k, b_k, w_v, b_v, w_o, b_o, _trace=False):
    from concourse.bass_utils import run_bass_kernel_spmd

    bf = ml_dtypes.bfloat16
    q = np.asarray(q, np.float32)
    k = np.asarray(k, np.float32)
    v = np.asarray(v, np.float32)
    w_q = np.asarray(w_q, np.float32)
    w_k = np.asarray(w_k, np.float32)
    w_v = np.asarray(w_v, np.float32)
    w_o = np.asarray(w_o, np.float32)
    b_q = np.asarray(b_q, np.float32)
    b_k = np.asarray(b_k, np.float32)
    b_v = np.asarray(b_v, np.float32)
    b_o = np.asarray(b_o, np.float32)

    nc = _get_nc()

    def tile_x(x):
        # [S, D] -> [NQT, 128, KC, QT]: A[nt, p, c, s] = x[nt*QT+s, c*128+p]
        t = x.T.reshape(KC, 128, NQT, QT)
        return np.ascontiguousarray(t.transpose(2, 1, 0, 3)).astype(bf)

    def tile_w(w, lo, hi):
        # [D, dloc] -> [128, KC, dloc]
        t = w[lo:hi, :].T.reshape(KC, 128, DLOC)
        return np.ascontiguousarray(t.transpose(1, 0, 2)).astype(bf)

    xqT = [tile_x(q[b]) for b in range(B)]
    xkT = [tile_x(k[b]) for b in range(B)]
    xvT = [tile_x(v[b]) for b in range(B)]

    in_maps = []
    for c in range(N_CORES):
        b, hg = c // 4, c % 4
        lo, hi = hg * DLOC, (hg + 1) * DLOC
        in_maps.append({
            "xq": xqT[b],
            "xk": xkT[b],
            "xv": xvT[b],
            "wq": tile_w(w_q, lo, hi),
            "wk": tile_w(w_k, lo, hi),
            "wv": tile_w(w_v, lo, hi),
            "wo": np.ascontiguousarray(
                w_o[:, lo:hi].T.reshape(2, 128, D).transpose(1, 0, 2)
            ).astype(bf),
            "bq": np.ascontiguousarray(b_q[lo:hi].reshape(2, 128).T),
            "bk": np.ascontiguousarray(b_k[lo:hi].reshape(2, 128).T),
        })

    res = run_bass_kernel_spmd(
        nc, in_maps, core_ids=list(range(N_CORES)), trace=_trace)
    if _trace:
        _CACHE["last_result"] = res

    # b_v contributes exactly (w_o @ b_v) per output element (softmax rows
    # sum to 1); b_o adds directly.
    const_row = (b_o + w_o @ b_v).astype(np.float32)  # [D]
    out = np.empty((B, S, D), np.float32)
    for b in range(B):
        acc = res.results[4 * b]["yT"].astype(np.float32)
        for c in range(4 * b + 1, 4 * b + 4):
            acc += res.results[c]["yT"].astype(np.float32)
        out[b] = acc.reshape(D, S).T + const_row
    return out



# revision 6
# speedup vs baseline: 1.0411x; 1.0411x over previous
"""Multi-head attention (B=2, S=2048, D=1024, H=16) on 8 Trainium2 NeuronCores.

Sharding: core c -> (batch b = c // 4, head-group hg = c % 4, 4 heads each).
Host sums the 4 partial output projections per batch and adds bias constants.

v7 changes vs v6:
- software-pipelined inner loop: AV matmuls for kt-1 are emitted after the
  scores pair for kt, so the in-order PE stream never waits on the current
  tile's exp (v6 stalled 0.5-2us per kt on $S[155]/$S[165] semaphores).
- exp is always split one head per engine per kt (ACT table exp + DVE
  Schraudolph bit-trick), no DVE-both kts; sums copy moved ACT->DVE.
- wq/wk host layout [128, 2(mt), KC, 128] so the DMA critical path is
  chunked per head-pair: K/Q-proj mt0 start after ~1/4 of the weight+x
  bytes instead of the full tensors (lead-in 14us -> ~10us).
- qt0 interleave reordered: only mt0 of K/Q is needed for hp0 scores, so
  mt1 projections move into the hp0 kt loop; V st2..15 JIT at kt-2.
- out-projection split into (pt, j) units spread over odd kts of hp1,
  casts alternating ACT/DVE; tail early-starts all 4 pairs' c0 half.
"""
import sys

sys.path.insert(0, "/opt/trn_rl_repo")

import numpy as np
import ml_dtypes

N_CORES = 8
B, S, D = 2, 2048, 1024
H, DH = 16, 64
DLOC = D // 4  # 256 head dims per core
QT = 512
NQT = S // QT  # 4
KT = 128
NKT = S // KT  # 16
KC = D // 128  # 8 contraction chunks

SCH_A = float(np.float32(128.0 * np.log2(np.e) / 8.0))
SCH_B = float(np.float32(127.0 * 128.0 - 5.5))
RECIP_MAGIC = 0x7EF477D5

_CACHE = {}


def _build():
    from concourse import bacc
    import concourse.mybir as mybir
    import concourse.tile as tile

    f32 = mybir.dt.float32
    bf16 = mybir.dt.bfloat16
    i16 = mybir.dt.int16
    i32 = mybir.dt.int32
    AF = mybir.ActivationFunctionType
    ALU = mybir.AluOpType

    nc = bacc.Bacc("TRN2", target_bir_lowering=False, debug=False,
                   num_devices=N_CORES)
    xq = nc.declare_dram_parameter("xq", [NQT, 128, KC, QT], bf16,
                                   isOutput=False)
    xk = nc.declare_dram_parameter("xk", [NQT, 128, KC, QT], bf16,
                                   isOutput=False)
    xv = nc.declare_dram_parameter("xv", [NQT, 128, KC, QT], bf16,
                                   isOutput=False)
    wq = nc.declare_dram_parameter("wq", [128, 2, KC, 128], bf16,
                                   isOutput=False)
    wk = nc.declare_dram_parameter("wk", [128, 2, KC, 128], bf16,
                                   isOutput=False)
    wv = nc.declare_dram_parameter("wv", [128, KC, DLOC], bf16, isOutput=False)
    wo = nc.declare_dram_parameter("wo", [128, 2, D], bf16, isOutput=False)
    bq = nc.declare_dram_parameter("bq", [128, 2], f32, isOutput=False)
    bk = nc.declare_dram_parameter("bk", [128, 2], f32, isOutput=False)
    yT = nc.declare_dram_parameter("yT", [8, 128, NQT, QT], bf16,
                                   isOutput=True)

    with tile.TileContext(nc) as tc:
        with (
            tc.tile_pool(name="keep", bufs=1) as keep,
            tc.tile_pool(name="big", bufs=1) as big,
            tc.tile_pool(name="xpool", bufs=6) as xpool,
            tc.tile_pool(name="esb", bufs=6) as esbp,
            tc.tile_pool(name="nrm", bufs=2) as nrmp,
            tc.tile_pool(name="ysb", bufs=2) as ysbp,
            tc.tile_pool(name="psS", bufs=4, space="PSUM") as psS,
            tc.tile_pool(name="psAV", bufs=2, space="PSUM") as psAV,
        ):
            # --- resident weights / constants ---
            wq_t = keep.tile([128, 2, KC, 128], bf16, tag="wq")
            wk_t = keep.tile([128, 2, KC, 128], bf16, tag="wk")
            wv_t = keep.tile([128, KC, DLOC], bf16, tag="wv")
            wo_t = keep.tile([128, 2, D], bf16, tag="wo")
            bq_t = keep.tile([128, 2], f32, tag="bq")
            bk_t = keep.tile([128, 2], f32, tag="bk")

            # persistent activations
            qh = big.tile([128, 2, S], bf16)   # [head-dim pair, hp, q]
            kh = big.tile([128, 2, S], bf16)
            vsb = big.tile([128, NKT, 4, 128], bf16)  # [kpos, kt, head, d|1s]
            aoT = big.tile([128, 2, S], bf16)  # attn out^T [dlocal, q]
            nc.vector.memset(vsb[:, :, :, 64:128], 1.0)

            # --- DMAs in critical-path order.  K/Q-proj mt0 gate the first
            # scores; mt1 and later x tiles arrive under the qt0 loop. ---
            def dma_w_mt(dst, src, mt):
                nc.sync.dma_start(out=dst[:, mt, 0:4, :], in_=src[:, mt, 0:4, :])
                nc.sync.dma_start(out=dst[:, mt, 4:8, :], in_=src[:, mt, 4:8, :])

            xtiles = {}

            def declare_x(kind, nt, chunks, axis="q"):
                x_d = {"q": xq, "k": xk, "v": xv}[kind]
                xc = xpool.tile([128, KC, QT], bf16, tag="xc")
                n = (KC if axis == "c" else QT) // chunks
                for ch in range(chunks):
                    sl = slice(ch * n, (ch + 1) * n)
                    if axis == "c":
                        nc.sync.dma_start(out=xc[:, sl, :],
                                          in_=x_d[nt, :, sl, :])
                    else:
                        nc.sync.dma_start(out=xc[:, :, sl],
                                          in_=x_d[nt, :, :, sl])
                xtiles[(kind, nt)] = xc

            nc.sync.dma_start(out=bk_t, in_=bk[:, :])
            dma_w_mt(wk_t, wk, 0)
            declare_x("k", 0, 2, axis="c")
            nc.sync.dma_start(out=bq_t, in_=bq[:, :])
            dma_w_mt(wq_t, wq, 0)
            declare_x("q", 0, 2, axis="c")
            nc.sync.dma_start(out=wv_t, in_=wv[:, :, :])
            declare_x("v", 0, 4)
            dma_w_mt(wk_t, wk, 1)
            dma_w_mt(wq_t, wq, 1)
            declare_x("k", 1, 2)
            declare_x("v", 1, 4)
            declare_x("k", 2, 2)
            declare_x("v", 2, 4)
            declare_x("k", 3, 2)
            declare_x("v", 3, 4)
            nc.sync.dma_start(out=wo_t, in_=wo[:, :, :])
            for nt in range(1, NQT):
                declare_x("q", nt, 2)

            def proj_qk_part(state, half):
                kind, nt, mt, o_t, w_t, b_t, ps = state
                xc = xtiles[(kind, nt)]
                for c in range(half * 4, half * 4 + 4):
                    nc.tensor.matmul(
                        ps,
                        w_t[:, mt, c, :],
                        xc[:, c, :],
                        start=(c == 0), stop=(c == KC - 1),
                        skip_group_check=True)
                if half == 1:
                    nc.scalar.activation(
                        o_t[:, mt, nt * QT:(nt + 1) * QT], ps,
                        AF.Identity, bias=b_t[:, mt:mt + 1])

            def proj_qk_mt(kind, nt, mt, o_t, w_t, b_t):
                ps = psS.tile([128, QT], f32, tag="sc")
                state = (kind, nt, mt, o_t, w_t, b_t, ps)
                proj_qk_part(state, 0)
                proj_qk_part(state, 1)

            def proj_v_st(st):
                # one 128-row chunk of sequence -> vsb[:, st, :, 0:64]
                nt, stl = st // 4, st % 4
                xc = xtiles[("v", nt)]
                ps = psS.tile([128, QT], f32, tag="sc")
                psv = ps[:, 0:DLOC]
                for c in range(KC):
                    nc.tensor.matmul(
                        psv,
                        xc[:, c, stl * 128:(stl + 1) * 128],
                        wv_t[:, c, :],
                        start=(c == 0), stop=(c == KC - 1),
                        skip_group_check=True)
                nc.scalar.copy(
                    vsb[:, st, :, 0:64],
                    psv.rearrange("p (h d) -> p h d", h=4))

            def out_proj_unit(qt, pt, j):
                # output rows [mt*128, (mt+1)*128) for q tile qt, mt=2pt+j
                qsl = slice(qt * QT, (qt + 1) * QT)
                py = psS.tile([128, QT], f32, tag="sc")
                mt = 2 * pt + j
                for c in range(2):
                    nc.tensor.matmul(
                        py,
                        wo_t[:, c, mt * 128:(mt + 1) * 128],
                        aoT[:, c, qsl],
                        start=(c == 0), stop=(c == 1),
                        skip_group_check=True)
                ysb = ysbp.tile([128, QT], bf16, tag="y")
                if j == 0:
                    nc.scalar.copy(ysb, py)
                else:
                    nc.vector.tensor_copy(ysb, py)
                nc.sync.dma_start(out=yT[mt, :, qt, :], in_=ysb)

            def norm_muls(av, rcp, hp, qsl):
                nc.vector.tensor_mul(
                    aoT[0:64, hp, qsl], av[0:64, 0, :], rcp[:, 0, :])
                nc.vector.tensor_mul(
                    aoT[64:128, hp, qsl], av[0:64, 1, :], rcp[:, 1, :])

            # --- interleave schedule: (qt, hp, kt) -> list of thunks ---
            sched = {}

            def at(qt, hp, kt, fn):
                sched.setdefault((qt, hp, kt), []).append(fn)

            def qk_half(kind, nt, mt, o_t, w_t, b_t, ctx, half):
                if half == 0:
                    ps = psS.tile([128, QT], f32, tag="sc")
                    ctx["s"] = (kind, nt, mt, o_t, w_t, b_t, ps)
                proj_qk_part(ctx["s"], half)

            # qt0 hp0: whole projections per slot so each x tile's readers
            # retire early (frees xpool slots for the in-flight DMA queue);
            # mt0 of nt lands before scores kt=4nt, mt1 before hp1.
            for kt0_slot, (kind, nt, mt) in (
                    (0, ("k", 0, 1)), (1, ("q", 0, 1)), (2, ("k", 1, 0)),
                    (4, ("k", 1, 1)), (5, ("k", 2, 0)), (6, ("k", 2, 1)),
                    (8, ("k", 3, 0)), (10, ("k", 3, 1))):
                o_t, w_t, b_t = ((kh, wk_t, bk_t) if kind == "k"
                                 else (qh, wq_t, bq_t))
                at(0, 0, kt0_slot,
                   (lambda k=kind, n=nt, m=mt, o=o_t, w=w_t, b=b_t:
                    proj_qk_mt(k, n, m, o, w, b)))
            at(0, 0, 0, lambda: proj_v_st(2))
            at(0, 0, 1, lambda: proj_v_st(3))
            for st in range(4, 16):
                at(0, 0, st - 1, lambda s=st: proj_v_st(s))
            # qt0 hp1: Q proj nt1 (mt0 at kts 2-3, mt1 at 9-10)
            for mt, k0 in ((0, 2), (1, 9)):
                ctx = {}
                for half in range(2):
                    at(0, 1, k0 + half,
                       (lambda c=ctx, m=mt, h=half:
                        qk_half("q", 1, m, qh, wq_t, bq_t, c, h)))
            # qt 1..2 hp0: Q proj qt+1 (mt0 at 6-7, mt1 at 12-13)
            for qt in range(1, NQT - 1):
                for mt, k0 in ((0, 6), (1, 12)):
                    ctx = {}
                    for half in range(2):
                        at(qt, 0, k0 + half,
                           (lambda c=ctx, n=qt + 1, m=mt, h=half:
                            qk_half("q", n, m, qh, wq_t, bq_t, c, h)))
            # qt >= 1 hp1: out projection of qt-1 at odd kts
            for qt in range(1, NQT):
                for u in range(8):
                    pt, j = u // 2, u % 2
                    at(qt, 1, 2 * u + 1,
                       lambda q=qt - 1, p=pt, jj=j: out_proj_unit(q, p, jj))

            # --- lead-in ---
            proj_qk_mt("k", 0, 0, kh, wk_t, bk_t)
            proj_qk_mt("q", 0, 0, qh, wq_t, bq_t)
            proj_v_st(0)
            proj_v_st(1)

            pending = None  # (av, hp, qsl) awaiting normalize
            ctx = {}

            # --- attention loop ---
            for qt in range(NQT):
                qsl = slice(qt * QT, (qt + 1) * QT)
                for hp in range(2):
                    av = psAV.tile([128, 2, QT], f32, tag="av")
                    esbs = {}
                    for kt in range(NKT):
                        ksl = slice(kt * 128, (kt + 1) * 128)
                        scA = psS.tile([128, QT], f32, tag="sc")
                        scB = psS.tile([128, QT], f32, tag="sc")
                        nc.tensor.matmul(
                            scA, kh[0:64, hp, ksl],
                            qh[0:64, hp, qsl], start=True, stop=True)
                        nc.tensor.matmul(
                            scB, kh[64:128, hp, ksl],
                            qh[64:128, hp, qsl], start=True, stop=True)
                        # interleaved work first: its PSUM->SBUF copies must
                        # precede this kt's exp in ACT/DVE program order, or
                        # the next scores' pool allocation deadlocks on them
                        for fn in sched.get((qt, hp, kt), ()):
                            fn()
                        esb = esbp.tile([128, 2, QT], bf16, tag="e")
                        dve_j = kt % 2
                        for j, scj in ((0, scA), (1, scB)):
                            if j == dve_j:
                                nc.vector.tensor_scalar(
                                    esb[:, j, :].bitcast(i16), scj,
                                    SCH_A, SCH_B, ALU.mult, ALU.add)
                            else:
                                nc.scalar.activation(
                                    esb[:, j, :], scj, AF.Exp, scale=0.125)
                        esbs[kt] = esb
                        # software pipeline: AV of kt-1
                        if kt > 0:
                            eprev = esbs.pop(kt - 1)
                            for j in range(2):
                                nc.tensor.matmul(
                                    av[:, j, :], vsb[:, kt - 1, 2 * hp + j, :],
                                    eprev[:, j, :],
                                    start=(kt - 1 == 0), stop=False,
                                    skip_group_check=True)
                        # deferred normalize chain for the previous segment
                        if pending is not None:
                            if kt == 0:
                                sums = nrmp.tile([64, 2, QT], f32, tag="sums")
                                nc.vector.tensor_copy(
                                    sums, pending[0][64:128, :, :])
                                ctx["sums"] = sums
                            elif kt == 1:
                                y0 = nrmp.tile([64, 2, QT], f32, tag="y0")
                                nc.gpsimd.tensor_scalar(
                                    y0[:].bitcast(i32),
                                    ctx["sums"][:].bitcast(i32),
                                    -1, RECIP_MAGIC, ALU.mult, ALU.add)
                                ctx["y0"] = y0
                            elif kt == 2:
                                m = nrmp.tile([64, 2, QT], f32, tag="m")
                                nc.gpsimd.tensor_mul(
                                    m, ctx["sums"], ctx["y0"])
                                ctx["m"] = m
                            elif kt == 3:
                                sn = nrmp.tile([64, 2, QT], f32, tag="s")
                                nc.gpsimd.tensor_scalar(
                                    sn, ctx["m"], -1.0, 2.0,
                                    ALU.mult, ALU.add)
                                ctx["s"] = sn
                            elif kt == 4:
                                rcp = nrmp.tile([64, 2, QT], f32, tag="rcp")
                                nc.gpsimd.tensor_mul(
                                    rcp, ctx["s"], ctx["y0"])
                                ctx["rcp"] = rcp
                            elif kt == 14:
                                norm_muls(pending[0], ctx["rcp"],
                                          pending[1], pending[2])
                                pending = None
                    # AV of kt 15
                    elast = esbs.pop(NKT - 1)
                    for j in range(2):
                        nc.tensor.matmul(
                            av[:, j, :], vsb[:, NKT - 1, 2 * hp + j, :],
                            elast[:, j, :],
                            start=False, stop=True,
                            skip_group_check=True)
                    pending = (av, hp, qsl)

            # tail: final normalize via exp(-ln); all 4 pairs start their
            # c=0 matmuls under the normalize latency
            qsl3 = slice((NQT - 1) * QT, NQT * QT)
            early = []
            for mt in range(4):
                py = psS.tile([128, QT], f32, tag="sc")
                nc.tensor.matmul(
                    py,
                    wo_t[:, 0, mt * 128:(mt + 1) * 128],
                    aoT[:, 0, qsl3],
                    start=True, stop=False, skip_group_check=True)
                early.append(py)
            lnt = nrmp.tile([64, 2, QT], f32, tag="m")
            nc.scalar.activation(lnt, pending[0][64:128, :, :], AF.Ln)
            rcp = nrmp.tile([64, 2, QT], f32, tag="rcp")
            nc.scalar.activation(rcp, lnt, AF.Exp, scale=-1.0)
            norm_muls(pending[0], rcp, pending[1], pending[2])
            for mt in range(4):
                py = early[mt]
                nc.tensor.matmul(
                    py,
                    wo_t[:, 1, mt * 128:(mt + 1) * 128],
                    aoT[:, 1, qsl3],
                    start=False, stop=True, skip_group_check=True)
                ysb = ysbp.tile([128, QT], bf16, tag="y")
                if mt % 2 == 0:
                    nc.scalar.copy(ysb, py)
                else:
                    nc.vector.tensor_copy(ysb, py)
                nc.sync.dma_start(out=yT[mt, :, NQT - 1, :], in_=ysb)
            for mt in range(4, 8):
                out_proj_unit(NQT - 1, mt // 2, mt % 2)
    nc.compile()
    return nc


def _get_nc():
    if "nc" not in _CACHE:
        _CACHE["nc"] = _build()
    return _CACHE["nc"]


def kernel(q, k, v, w_q, b_q, w_k, b_k, w_v, b_v, w_o, b_o, _trace=False):
    from concourse.bass_utils import run_bass_kernel_spmd

    bf = ml_dtypes.bfloat16
    q = np.asarray(q, np.float32)
    k = np.asarray(k, np.float32)
    v = np.asarray(v, np.float32)
    w_q = np.asarray(w_q, np.float32)
    w_k = np.asarray(w_k, np.float32)
    w_v = np.asarray(w_v, np.float32)
    w_o = np.asarray(w_o, np.float32)
    b_q = np.asarray(b_q, np.float32)
    b_k = np.asarray(b_k, np.float32)
    b_v = np.asarray(b_v, np.float32)
    b_o = np.asarray(b_o, np.float32)

    nc = _get_nc()

    def tile_x(x):
        # [S, D] -> [NQT, 128, KC, QT]: A[nt, p, c, s] = x[nt*QT+s, c*128+p]
        t = x.T.reshape(KC, 128, NQT, QT)
        return np.ascontiguousarray(t.transpose(2, 1, 0, 3)).astype(bf)

    def tile_w_qk(w, lo, hi):
        # [D, dloc] -> [128, 2, KC, 128]: W[p, mt, c, m] = w[lo+mt*128+m, c*128+p]
        t = w[lo:hi, :].T.reshape(KC, 128, 2, 128)
        return np.ascontiguousarray(t.transpose(1, 2, 0, 3)).astype(bf)

    def tile_w_v(w, lo, hi):
        # [D, dloc] -> [128, KC, dloc]
        t = w[lo:hi, :].T.reshape(KC, 128, DLOC)
        return np.ascontiguousarray(t.transpose(1, 0, 2)).astype(bf)

    xqT = [tile_x(q[b]) for b in range(B)]
    xkT = [tile_x(k[b]) for b in range(B)]
    xvT = [tile_x(v[b]) for b in range(B)]

    in_maps = []
    for c in range(N_CORES):
        b, hg = c // 4, c % 4
        lo, hi = hg * DLOC, (hg + 1) * DLOC
        in_maps.append({
            "xq": xqT[b],
            "xk": xkT[b],
            "xv": xvT[b],
            "wq": tile_w_qk(w_q, lo, hi),
            "wk": tile_w_qk(w_k, lo, hi),
            "wv": tile_w_v(w_v, lo, hi),
            "wo": np.ascontiguousarray(
                w_o[:, lo:hi].T.reshape(2, 128, D).transpose(1, 0, 2)
            ).astype(bf),
            "bq": np.ascontiguousarray(b_q[lo:hi].reshape(2, 128).T),
            "bk": np.ascontiguousarray(b_k[lo:hi].reshape(2, 128).T),
        })

    res = run_bass_kernel_spmd(
        nc, in_maps, core_ids=list(range(N_CORES)), trace=_trace)
    if _trace:
        _CACHE["last_result"] = res

    # b_v contributes exactly (w_o @ b_v) per output element (softmax rows
    # sum to 1); b_o adds directly.
    const_row = (b_o + w_o @ b_v).astype(np.float32)  # [D]
    out = np.empty((B, S, D), np.float32)
    for b in range(B):
        acc = res.results[4 * b]["yT"].astype(np.float32)
        for c in range(4 * b + 1, 4 * b + 4):
            acc += res.results[c]["yT"].astype(np.float32)
        out[b] = acc.reshape(D, S).T + const_row
    return out


# revision 11
# speedup vs baseline: 1.0448x; 1.0036x over previous
"""Multi-head attention (B=2, S=2048, D=1024, H=16) on 8 Trainium2 NeuronCores.

Sharding: core c -> (batch b = c // 4, head-group hg = c % 4, 4 heads each).
Host sums the 4 partial output projections per batch and adds bias constants.

v7 changes vs v6:
- software-pipelined inner loop: AV matmuls for kt-1 are emitted after the
  scores pair for kt, so the in-order PE stream never waits on the current
  tile's exp (v6 stalled 0.5-2us per kt on $S[155]/$S[165] semaphores).
- exp is always split one head per engine per kt (ACT table exp + DVE
  Schraudolph bit-trick), no DVE-both kts; sums copy moved ACT->DVE.
- wq/wk host layout [128, 2(mt), KC, 128] so the DMA critical path is
  chunked per head-pair: K/Q-proj mt0 start after ~1/4 of the weight+x
  bytes instead of the full tensors (lead-in 14us -> ~10us).
- qt0 interleave reordered: only mt0 of K/Q is needed for hp0 scores, so
  mt1 projections move into the hp0 kt loop; V st2..15 JIT at kt-2.
- out-projection split into (pt, j) units spread over odd kts of hp1,
  casts alternating ACT/DVE; tail early-starts all 4 pairs' c0 half.
"""
import sys

sys.path.insert(0, "/opt/trn_rl_repo")

import numpy as np
import ml_dtypes

N_CORES = 8
B, S, D = 2, 2048, 1024
H, DH = 16, 64
DLOC = D // 4  # 256 head dims per core
QT = 512
NQT = S // QT  # 4
KT = 128
NKT = S // KT  # 16
KC = D // 128  # 8 contraction chunks

SCH_A = float(np.float32(128.0 * np.log2(np.e) / 8.0))
SCH_B = float(np.float32(127.0 * 128.0 - 5.5))
RECIP_MAGIC = 0x7EF477D5

_CACHE = {}


def _build():
    from concourse import bacc
    import concourse.mybir as mybir
    import concourse.tile as tile

    f32 = mybir.dt.float32
    bf16 = mybir.dt.bfloat16
    i16 = mybir.dt.int16
    i32 = mybir.dt.int32
    AF = mybir.ActivationFunctionType
    ALU = mybir.AluOpType

    nc = bacc.Bacc("TRN2", target_bir_lowering=False, debug=False,
                   num_devices=N_CORES)
    xq = nc.declare_dram_parameter("xq", [NQT, 128, KC, QT], bf16,
                                   isOutput=False)
    xk = nc.declare_dram_parameter("xk", [NQT, 128, KC, QT], bf16,
                                   isOutput=False)
    xv = nc.declare_dram_parameter("xv", [NQT, 128, KC, QT], bf16,
                                   isOutput=False)
    wq = nc.declare_dram_parameter("wq", [128, 2, KC, 128], bf16,
                                   isOutput=False)
    wk = nc.declare_dram_parameter("wk", [128, 2, KC, 128], bf16,
                                   isOutput=False)
    wv = nc.declare_dram_parameter("wv", [128, KC, DLOC], bf16, isOutput=False)
    wo = nc.declare_dram_parameter("wo", [128, 2, D], bf16, isOutput=False)
    bq = nc.declare_dram_parameter("bq", [128, 2], f32, isOutput=False)
    bk = nc.declare_dram_parameter("bk", [128, 2], f32, isOutput=False)
    yT = nc.declare_dram_parameter("yT", [8, 128, NQT, QT], bf16,
                                   isOutput=True)

    with tile.TileContext(nc) as tc:
        with (
            tc.tile_pool(name="keep", bufs=1) as keep,
            tc.tile_pool(name="big", bufs=1) as big,
            tc.tile_pool(name="xpool", bufs=6) as xpool,
            tc.tile_pool(name="esbA", bufs=6) as esbAp,
            tc.tile_pool(name="esbD", bufs=6) as esbDp,
            tc.tile_pool(name="nrm", bufs=2) as nrmp,
            tc.tile_pool(name="ysb", bufs=2) as ysbp,
            tc.tile_pool(name="psS", bufs=4, space="PSUM") as psS,
            tc.tile_pool(name="psAV", bufs=2, space="PSUM") as psAV,
        ):
            # --- resident weights / constants ---
            wq_t = keep.tile([128, 2, KC, 128], bf16, tag="wq")
            wk_t = keep.tile([128, 2, KC, 128], bf16, tag="wk")
            wv_t = keep.tile([128, KC, DLOC], bf16, tag="wv")
            wo_t = keep.tile([128, 2, D], bf16, tag="wo")
            bq_t = keep.tile([128, 2], f32, tag="bq")
            bk_t = keep.tile([128, 2], f32, tag="bk")

            # persistent activations
            qh = big.tile([128, 2, S], bf16)   # [head-dim pair, hp, q]
            kh = big.tile([128, 2, S], bf16)
            vsb = big.tile([128, NKT, 4, 128], bf16)  # [kpos, kt, head, d|1s]
            aoT = big.tile([128, 2, S], bf16)  # attn out^T [dlocal, q]
            nc.vector.memset(vsb[:, :, :, 64:128], 1.0)

            # --- DMAs in critical-path order.  K/Q-proj mt0 gate the first
            # scores; mt1 and later x tiles arrive under the qt0 loop. ---
            def dma_w_mt(dst, src, mt):
                nc.sync.dma_start(out=dst[:, mt, 0:4, :], in_=src[:, mt, 0:4, :])
                nc.sync.dma_start(out=dst[:, mt, 4:8, :], in_=src[:, mt, 4:8, :])

            xtiles = {}

            def declare_x(kind, nt, chunks, axis="q"):
                x_d = {"q": xq, "k": xk, "v": xv}[kind]
                xc = xpool.tile([128, KC, QT], bf16, tag="xc")
                n = (KC if axis == "c" else QT) // chunks
                for ch in range(chunks):
                    sl = slice(ch * n, (ch + 1) * n)
                    if axis == "c":
                        nc.sync.dma_start(out=xc[:, sl, :],
                                          in_=x_d[nt, :, sl, :])
                    else:
                        nc.sync.dma_start(out=xc[:, :, sl],
                                          in_=x_d[nt, :, :, sl])
                xtiles[(kind, nt)] = xc

            nc.sync.dma_start(out=bk_t, in_=bk[:, :])
            dma_w_mt(wk_t, wk, 0)
            declare_x("k", 0, 2, axis="c")
            nc.sync.dma_start(out=bq_t, in_=bq[:, :])
            dma_w_mt(wq_t, wq, 0)
            declare_x("q", 0, 2, axis="c")
            nc.sync.dma_start(out=wv_t, in_=wv[:, :, :])
            declare_x("v", 0, 4)
            dma_w_mt(wk_t, wk, 1)
            dma_w_mt(wq_t, wq, 1)
            declare_x("k", 1, 2)
            declare_x("v", 1, 4)
            declare_x("k", 2, 2)
            declare_x("v", 2, 4)
            declare_x("k", 3, 2)
            declare_x("v", 3, 4)
            nc.sync.dma_start(out=wo_t, in_=wo[:, :, :])
            for nt in range(1, NQT):
                declare_x("q", nt, 2)

            def proj_qk_mm(state, half):
                kind, nt, mt, o_t, w_t, b_t, ps = state
                xc = xtiles[(kind, nt)]
                for c in range(half * 4, half * 4 + 4):
                    nc.tensor.matmul(
                        ps,
                        w_t[:, mt, c, :],
                        xc[:, c, :],
                        start=(c == 0), stop=(c == KC - 1),
                        skip_group_check=True)

            def proj_qk_copy(state):
                kind, nt, mt, o_t, w_t, b_t, ps = state
                nc.scalar.activation(
                    o_t[:, mt, nt * QT:(nt + 1) * QT], ps,
                    AF.Identity, bias=b_t[:, mt:mt + 1])

            def proj_qk_mt(kind, nt, mt, o_t, w_t, b_t):
                ps = psS.tile([128, QT], f32, tag="sc")
                state = (kind, nt, mt, o_t, w_t, b_t, ps)
                proj_qk_mm(state, 0)
                proj_qk_mm(state, 1)
                proj_qk_copy(state)

            def proj_v_mm(st, vctx):
                # one 128-row chunk of sequence -> vsb[:, st, :, 0:64]
                nt, stl = st // 4, st % 4
                xc = xtiles[("v", nt)]
                ps = psS.tile([128, QT], f32, tag="sc")
                psv = ps[:, 0:DLOC]
                for c in range(KC):
                    nc.tensor.matmul(
                        psv,
                        xc[:, c, stl * 128:(stl + 1) * 128],
                        wv_t[:, c, :],
                        start=(c == 0), stop=(c == KC - 1),
                        skip_group_check=True)
                vctx[st] = psv

            def proj_v_copy(st, vctx):
                nc.scalar.copy(
                    vsb[:, st, :, 0:64],
                    vctx.pop(st).rearrange("p (h d) -> p h d", h=4))

            def proj_v_st(st):
                vctx = {}
                proj_v_mm(st, vctx)
                proj_v_copy(st, vctx)

            def out_mm(qt, mt, octx):
                qsl = slice(qt * QT, (qt + 1) * QT)
                py = psS.tile([128, QT], f32, tag="sc")
                for c in range(2):
                    nc.tensor.matmul(
                        py,
                        wo_t[:, c, mt * 128:(mt + 1) * 128],
                        aoT[:, c, qsl],
                        start=(c == 0), stop=(c == 1),
                        skip_group_check=True)
                octx[(qt, mt)] = py

            def out_copy(qt, mt, octx):
                py = octx.pop((qt, mt))
                ysb = ysbp.tile([128, QT], bf16, tag="y")
                if mt % 2 == 0:
                    nc.scalar.copy(ysb, py)
                else:
                    nc.vector.tensor_copy(ysb, py)
                nc.sync.dma_start(out=yT[mt, :, qt, :], in_=ysb)

            def out_proj_unit(qt, pt, j):
                octx = {}
                out_mm(qt, 2 * pt + j, octx)
                out_copy(qt, 2 * pt + j, octx)

            def norm_muls(av, rcp, hp, qsl):
                nc.vector.tensor_mul(
                    aoT[0:64, hp, qsl], av[0:64, 0, :], rcp[:, 0, :])
                nc.vector.tensor_mul(
                    aoT[64:128, hp, qsl], av[0:64, 1, :], rcp[:, 1, :])

            # --- interleave schedule: (qt, hp, kt) -> list of thunks.
            # Matmul thunks run at slot kt; the PSUM->SBUF copy runs at a
            # LATER kt, registered at the list head so it precedes that kt's
            # exp on ACT/DVE (its deps are then already satisfied: no
            # head-of-line blocking, and pool recycling stays acyclic). ---
            sched = {}

            def at(qt, hp, kt, fn, front=False):
                lst = sched.setdefault((qt, hp, kt), [])
                lst.insert(0, fn) if front else lst.append(fn)

            def sched_qk(qt, hp, kind, nt, mt, kts, copy_kt):
                o_t, w_t, b_t = ((kh, wk_t, bk_t) if kind == "k"
                                 else (qh, wq_t, bq_t))
                ctx = {}

                def mk_mm(half):
                    def fn():
                        if "s" not in ctx:
                            ps = psS.tile([128, QT], f32, tag="sc")
                            ctx["s"] = (kind, nt, mt, o_t, w_t, b_t, ps)
                        proj_qk_mm(ctx["s"], half)
                    return fn

                if len(kts) == 1:
                    def both():
                        mk_mm(0)()
                        mk_mm(1)()
                    at(qt, hp, kts[0], both)
                else:
                    at(qt, hp, kts[0], mk_mm(0))
                    at(qt, hp, kts[1], mk_mm(1))
                at(qt, hp, copy_kt, lambda: proj_qk_copy(ctx["s"]),
                   front=True)

            # qt0 hp0: projections per slot so each x tile's readers retire
            # early (frees xpool slots for the in-flight DMA queue); mt0 of
            # nt lands before scores kt=4nt, mt1 before hp1.
            for slot, (kind, nt, mt) in (
                    (0, ("k", 0, 1)), (1, ("q", 0, 1)), (2, ("k", 1, 0)),
                    (4, ("k", 1, 1)), (5, ("k", 2, 0)), (6, ("k", 2, 1)),
                    (8, ("k", 3, 0)), (10, ("k", 3, 1))):
                sched_qk(0, 0, kind, nt, mt, [slot], slot + 1)
            vctx = {}
            at(0, 0, 0, lambda: proj_v_mm(2, vctx))
            at(0, 0, 1, lambda: proj_v_mm(3, vctx))
            at(0, 0, 1, lambda: proj_v_copy(2, vctx), front=True)
            at(0, 0, 2, lambda: proj_v_copy(3, vctx), front=True)
            for st in range(4, 16):
                at(0, 0, st - 1, lambda s=st: proj_v_mm(s, vctx))
                at(0, 0, st, lambda s=st: proj_v_copy(s, vctx), front=True)
            # qt0 hp1: Q proj nt1
            sched_qk(0, 1, "q", 1, 0, [2, 3], 4)
            sched_qk(0, 1, "q", 1, 1, [9, 10], 11)
            # qt 1..2 hp0: Q proj qt+1
            for qt in range(1, NQT - 1):
                sched_qk(qt, 0, "q", qt + 1, 0, [6, 7], 8)
                sched_qk(qt, 0, "q", qt + 1, 1, [12, 13], 14)
            # qt >= 1 hp1: out projection of qt-1, mm at even kts,
            # cast+DMA at the following odd kt
            for qt in range(1, NQT):
                octx = {}
                for mt in range(8):
                    at(qt, 1, 2 * mt,
                       lambda q=qt - 1, m=mt, o=octx: out_mm(q, m, o))
                    at(qt, 1, 2 * mt + 1,
                       lambda q=qt - 1, m=mt, o=octx: out_copy(q, m, o),
                       front=True)

            # --- lead-in ---
            proj_qk_mt("k", 0, 0, kh, wk_t, bk_t)
            proj_qk_mt("q", 0, 0, qh, wq_t, bq_t)
            proj_v_st(0)
            proj_v_st(1)

            pending = None  # (av, hp, qsl) awaiting normalize
            ctx = {}

            # --- attention loop ---
            for qt in range(NQT):
                qsl = slice(qt * QT, (qt + 1) * QT)
                for hp in range(2):
                    av = psAV.tile([128, 2, QT], f32, tag="av")
                    esbs = {}
                    for kt in range(NKT):
                        ksl = slice(kt * 128, (kt + 1) * 128)
                        scA = psS.tile([128, QT], f32, tag="sc")
                        scB = psS.tile([128, QT], f32, tag="sc")
                        nc.tensor.matmul(
                            scA, kh[0:64, hp, ksl],
                            qh[0:64, hp, qsl], start=True, stop=True)
                        nc.tensor.matmul(
                            scB, kh[64:128, hp, ksl],
                            qh[64:128, hp, qsl], start=True, stop=True)
                        # interleaved work first: its PSUM->SBUF copies must
                        # precede this kt's exp in ACT/DVE program order, or
                        # the next scores' pool allocation deadlocks on them
                        for fn in sched.get((qt, hp, kt), ()):
                            fn()
                        # per-engine esb pools: each engine only ever rewrites
                        # its own buffers, so recycling needs no cross-engine
                        # write-after-write semaphores
                        dve_j = kt % 2
                        eD = esbDp.tile([128, QT], bf16, tag="eD")
                        eA = esbAp.tile([128, QT], bf16, tag="eA")
                        nc.vector.tensor_scalar(
                            eD[:].bitcast(i16), scA if dve_j == 0 else scB,
                            SCH_A, SCH_B, ALU.mult, ALU.add)
                        nc.scalar.activation(
                            eA, scB if dve_j == 0 else scA,
                            AF.Exp, scale=0.125)
                        esbs[kt] = (eA, eD, dve_j)
                        # software pipeline: AV of kt-1
                        if kt > 0:
                            eA1, eD1, dj1 = esbs.pop(kt - 1)
                            for j in range(2):
                                nc.tensor.matmul(
                                    av[:, j, :], vsb[:, kt - 1, 2 * hp + j, :],
                                    eD1 if j == dj1 else eA1,
                                    start=(kt - 1 == 0), stop=False,
                                    skip_group_check=True)
                        # deferred normalize chain for the previous segment
                        if pending is not None:
                            if kt == 0:
                                sums = nrmp.tile([64, 2, QT], f32, tag="sums")
                                nc.vector.tensor_copy(
                                    sums, pending[0][64:128, :, :])
                                ctx["sums"] = sums
                            elif kt == 1:
                                y0 = nrmp.tile([64, 2, QT], f32, tag="y0")
                                nc.gpsimd.tensor_scalar(
                                    y0[:].bitcast(i32),
                                    ctx["sums"][:].bitcast(i32),
                                    -1, RECIP_MAGIC, ALU.mult, ALU.add)
                                ctx["y0"] = y0
                            elif kt == 2:
                                m = nrmp.tile([64, 2, QT], f32, tag="m")
                                nc.gpsimd.tensor_mul(
                                    m, ctx["sums"], ctx["y0"])
                                ctx["m"] = m
                            elif kt == 3:
                                sn = nrmp.tile([64, 2, QT], f32, tag="s")
                                nc.gpsimd.tensor_scalar(
                                    sn, ctx["m"], -1.0, 2.0,
                                    ALU.mult, ALU.add)
                                ctx["s"] = sn
                            elif kt == 4:
                                rcp = nrmp.tile([64, 2, QT], f32, tag="rcp")
                                nc.gpsimd.tensor_mul(
                                    rcp, ctx["s"], ctx["y0"])
                                ctx["rcp"] = rcp
                            elif kt == 14:
                                norm_muls(pending[0], ctx["rcp"],
                                          pending[1], pending[2])
                                pending = None
                    # AV of kt 15
                    eA1, eD1, dj1 = esbs.pop(NKT - 1)
                    for j in range(2):
                        nc.tensor.matmul(
                            av[:, j, :], vsb[:, NKT - 1, 2 * hp + j, :],
                            eD1 if j == dj1 else eA1,
                            start=False, stop=True,
                            skip_group_check=True)
                    pending = (av, hp, qsl)

            # tail: final normalize via exp(-ln); all 4 pairs start their
            # c=0 matmuls under the normalize latency
            qsl3 = slice((NQT - 1) * QT, NQT * QT)
            early = []
            for mt in range(4):
                py = psS.tile([128, QT], f32, tag="sc")
                nc.tensor.matmul(
                    py,
                    wo_t[:, 0, mt * 128:(mt + 1) * 128],
                    aoT[:, 0, qsl3],
                    start=True, stop=False, skip_group_check=True)
                early.append(py)
            lnt = nrmp.tile([64, 2, QT], f32, tag="m")
            nc.scalar.activation(lnt, pending[0][64:128, :, :], AF.Ln)
            rcp = nrmp.tile([64, 2, QT], f32, tag="rcp")
            nc.scalar.activation(rcp, lnt, AF.Exp, scale=-1.0)
            norm_muls(pending[0], rcp, pending[1], pending[2])
            for mt in range(4):
                py = early[mt]
                nc.tensor.matmul(
                    py,
                    wo_t[:, 1, mt * 128:(mt + 1) * 128],
                    aoT[:, 1, qsl3],
                    start=False, stop=True, skip_group_check=True)
                ysb = ysbp.tile([128, QT], bf16, tag="y")
                if mt % 2 == 0:
                    nc.scalar.copy(ysb, py)
                else:
                    nc.vector.tensor_copy(ysb, py)
                nc.sync.dma_start(out=yT[mt, :, NQT - 1, :], in_=ysb)
            for mt in range(4, 8):
                out_proj_unit(NQT - 1, mt // 2, mt % 2)
    nc.compile()
    return nc


def _get_nc():
    if "nc" not in _CACHE:
        _CACHE["nc"] = _build()
    return _CACHE["nc"]


def kernel(q, k, v, w_q, b_q, w_k, b_k, w_v, b_v, w_o, b_o, _trace=False):
    from concourse.bass_utils import run_bass_kernel_spmd

    bf = ml_dtypes.bfloat16
    q = np.asarray(q, np.float32)
    k = np.asarray(k, np.float32)
    v = np.asarray(v, np.float32)
    w_q = np.asarray(w_q, np.float32)
    w_k = np.asarray(w_k, np.float32)
    w_v = np.asarray(w_v, np.float32)
    w_o = np.asarray(w_o, np.float32)
    b_q = np.asarray(b_q, np.float32)
    b_k = np.asarray(b_k, np.float32)
    b_v = np.asarray(b_v, np.float32)
    b_o = np.asarray(b_o, np.float32)

    nc = _get_nc()

    def tile_x(x):
        # [S, D] -> [NQT, 128, KC, QT]: A[nt, p, c, s] = x[nt*QT+s, c*128+p]
        t = x.T.reshape(KC, 128, NQT, QT)
        return np.ascontiguousarray(t.transpose(2, 1, 0, 3)).astype(bf)

    def tile_w_qk(w, lo, hi):
        # [D, dloc] -> [128, 2, KC, 128]: W[p, mt, c, m] = w[lo+mt*128+m, c*128+p]
        t = w[lo:hi, :].T.reshape(KC, 128, 2, 128)
        return np.ascontiguousarray(t.transpose(1, 2, 0, 3)).astype(bf)

    def tile_w_v(w, lo, hi):
        # [D, dloc] -> [128, KC, dloc]
        t = w[lo:hi, :].T.reshape(KC, 128, DLOC)
        return np.ascontiguousarray(t.transpose(1, 0, 2)).astype(bf)

    xqT = [tile_x(q[b]) for b in range(B)]
    xkT = [tile_x(k[b]) for b in range(B)]
    xvT = [tile_x(v[b]) for b in range(B)]

    in_maps = []
    for c in range(N_CORES):
        b, hg = c // 4, c % 4
        lo, hi = hg * DLOC, (hg + 1) * DLOC
        in_maps.append({
            "xq": xqT[b],
            "xk": xkT[b],
            "xv": xvT[b],
            "wq": tile_w_qk(w_q, lo, hi),
            "wk": tile_w_qk(w_k, lo, hi),
            "wv": tile_w_v(w_v, lo, hi),
            "wo": np.ascontiguousarray(
                w_o[:, lo:hi].T.reshape(2, 128, D).transpose(1, 0, 2)
            ).astype(bf),
            "bq": np.ascontiguousarray(b_q[lo:hi].reshape(2, 128).T),
            "bk": np.ascontiguousarray(b_k[lo:hi].reshape(2, 128).T),
        })

    res = run_bass_kernel_spmd(
        nc, in_maps, core_ids=list(range(N_CORES)), trace=_trace)
    if _trace:
        _CACHE["last_result"] = res

    # b_v contributes exactly (w_o @ b_v) per output element (softmax rows
    # sum to 1); b_o adds directly.
    const_row = (b_o + w_o @ b_v).astype(np.float32)  # [D]
    out = np.empty((B, S, D), np.float32)
    for b in range(B):
        acc = res.results[4 * b]["yT"].astype(np.float32)
        for c in range(4 * b + 1, 4 * b + 4):
            acc += res.results[c]["yT"].astype(np.float32)
        out[b] = acc.reshape(D, S).T + const_row
    return out


# revision 13
# speedup vs baseline: 1.0992x; 1.0521x over previous
"""Multi-head attention (B=2, S=2048, D=1024, H=16) on 8 Trainium2 NeuronCores.

Sharding: core c -> (batch b = c // 4, head-group hg = c % 4, 4 heads each).
Host sums the 4 partial output projections per batch and adds bias constants.

v7 changes vs v6:
- software-pipelined inner loop: AV matmuls for kt-1 are emitted after the
  scores pair for kt, so the in-order PE stream never waits on the current
  tile's exp (v6 stalled 0.5-2us per kt on $S[155]/$S[165] semaphores).
- exp is always split one head per engine per kt (ACT table exp + DVE
  Schraudolph bit-trick), no DVE-both kts; sums copy moved ACT->DVE.
- wq/wk host layout [128, 2(mt), KC, 128] so the DMA critical path is
  chunked per head-pair: K/Q-proj mt0 start after ~1/4 of the weight+x
  bytes instead of the full tensors (lead-in 14us -> ~10us).
- qt0 interleave reordered: only mt0 of K/Q is needed for hp0 scores, so
  mt1 projections move into the hp0 kt loop; V st2..15 JIT at kt-2.
- out-projection split into (pt, j) units spread over odd kts of hp1,
  casts alternating ACT/DVE; tail early-starts all 4 pairs' c0 half.
"""
import sys

sys.path.insert(0, "/opt/trn_rl_repo")

import numpy as np
import ml_dtypes

N_CORES = 8
B, S, D = 2, 2048, 1024
H, DH = 16, 64
DLOC = D // 4  # 256 head dims per core
QT = 512
NQT = S // QT  # 4
KT = 128
NKT = S // KT  # 16
KC = D // 128  # 8 contraction chunks

SCH_A = float(np.float32(128.0 * np.log2(np.e) / 8.0))
SCH_B = float(np.float32(127.0 * 128.0 - 5.5))
RECIP_MAGIC = 0x7EF477D5

_CACHE = {}


def _build():
    from concourse import bacc
    import concourse.mybir as mybir
    import concourse.tile as tile

    f32 = mybir.dt.float32
    bf16 = mybir.dt.bfloat16
    i16 = mybir.dt.int16
    i32 = mybir.dt.int32
    AF = mybir.ActivationFunctionType
    ALU = mybir.AluOpType

    nc = bacc.Bacc("TRN2", target_bir_lowering=False, debug=False,
                   num_devices=N_CORES)
    xq = nc.declare_dram_parameter("xq", [NQT, 128, KC, QT], bf16,
                                   isOutput=False)
    xk = nc.declare_dram_parameter("xk", [NQT, 128, KC, QT], bf16,
                                   isOutput=False)
    xv = nc.declare_dram_parameter("xv", [NQT, 128, KC, QT], bf16,
                                   isOutput=False)
    wq = nc.declare_dram_parameter("wq", [128, 2, KC, 128], bf16,
                                   isOutput=False)
    wk = nc.declare_dram_parameter("wk", [128, 2, KC, 128], bf16,
                                   isOutput=False)
    wv = nc.declare_dram_parameter("wv", [128, KC, DLOC], bf16, isOutput=False)
    wo = nc.declare_dram_parameter("wo", [128, 2, D], bf16, isOutput=False)
    bq = nc.declare_dram_parameter("bq", [128, 2], f32, isOutput=False)
    bk = nc.declare_dram_parameter("bk", [128, 2], f32, isOutput=False)
    yT = nc.declare_dram_parameter("yT", [8, 128, NQT, QT], bf16,
                                   isOutput=True)

    with tile.TileContext(nc) as tc:
        with (
            tc.tile_pool(name="keep", bufs=1) as keep,
            tc.tile_pool(name="big", bufs=1) as big,
            tc.tile_pool(name="xpool", bufs=6) as xpool,
            tc.tile_pool(name="esbA", bufs=6) as esbAp,
            tc.tile_pool(name="esbD", bufs=6) as esbDp,
            tc.tile_pool(name="nrm", bufs=2) as nrmp,
            tc.tile_pool(name="ysb", bufs=2) as ysbp,
            tc.tile_pool(name="psS", bufs=4, space="PSUM") as psS,
            tc.tile_pool(name="psAV", bufs=2, space="PSUM") as psAV,
        ):
            # --- resident weights / constants ---
            wq_t = keep.tile([128, 2, KC, 128], bf16, tag="wq")
            wk_t = keep.tile([128, 2, KC, 128], bf16, tag="wk")
            wv_t = keep.tile([128, KC, DLOC], bf16, tag="wv")
            wo_t = keep.tile([128, 2, D], bf16, tag="wo")
            bq_t = keep.tile([128, 2], f32, tag="bq")
            bk_t = keep.tile([128, 2], f32, tag="bk")

            # persistent activations
            qh = big.tile([128, 2, S], bf16)   # [head-dim pair, hp, q]
            kh = big.tile([128, 2, S], bf16)
            vsb = big.tile([128, NKT, 4, 128], bf16)  # [kpos, kt, head, d|1s]
            aoT = big.tile([128, 2, S], bf16)  # attn out^T [dlocal, q]
            nc.vector.memset(vsb[:, :, :, 64:128], 1.0)

            # --- DMAs in critical-path order.  K/Q-proj mt0 gate the first
            # scores; mt1 and later x tiles arrive under the qt0 loop. ---
            def dma_w_mt(dst, src, mt):
                nc.sync.dma_start(out=dst[:, mt, 0:4, :], in_=src[:, mt, 0:4, :])
                nc.sync.dma_start(out=dst[:, mt, 4:8, :], in_=src[:, mt, 4:8, :])

            xtiles = {}

            def declare_x(kind, nt, chunks, axis="q"):
                x_d = {"q": xq, "k": xk, "v": xv}[kind]
                xc = xpool.tile([128, KC, QT], bf16, tag="xc")
                n = (KC if axis == "c" else QT) // chunks
                for ch in range(chunks):
                    sl = slice(ch * n, (ch + 1) * n)
                    if axis == "c":
                        nc.sync.dma_start(out=xc[:, sl, :],
                                          in_=x_d[nt, :, sl, :])
                    else:
                        nc.sync.dma_start(out=xc[:, :, sl],
                                          in_=x_d[nt, :, :, sl])
                xtiles[(kind, nt)] = xc

            nc.sync.dma_start(out=bk_t, in_=bk[:, :])
            dma_w_mt(wk_t, wk, 0)
            declare_x("k", 0, 2, axis="c")
            nc.sync.dma_start(out=bq_t, in_=bq[:, :])
            dma_w_mt(wq_t, wq, 0)
            declare_x("q", 0, 2, axis="c")
            nc.sync.dma_start(out=wv_t, in_=wv[:, :, :])
            declare_x("v", 0, 4)
            dma_w_mt(wk_t, wk, 1)
            dma_w_mt(wq_t, wq, 1)
            declare_x("k", 1, 2)
            declare_x("v", 1, 4)
            declare_x("k", 2, 2)
            declare_x("v", 2, 4)
            declare_x("k", 3, 2)
            declare_x("v", 3, 4)
            nc.sync.dma_start(out=wo_t, in_=wo[:, :, :])
            for nt in range(1, NQT):
                declare_x("q", nt, 2)

            def proj_qk_mm(state, half):
                kind, nt, mt, o_t, w_t, b_t, ps = state
                xc = xtiles[(kind, nt)]
                for c in range(half * 4, half * 4 + 4):
                    nc.tensor.matmul(
                        ps,
                        w_t[:, mt, c, :],
                        xc[:, c, :],
                        start=(c == 0), stop=(c == KC - 1),
                        skip_group_check=True)

            def proj_qk_copy(state):
                kind, nt, mt, o_t, w_t, b_t, ps = state
                nc.scalar.activation(
                    o_t[:, mt, nt * QT:(nt + 1) * QT], ps,
                    AF.Identity, bias=b_t[:, mt:mt + 1])

            def proj_qk_mt(kind, nt, mt, o_t, w_t, b_t):
                ps = psS.tile([128, QT], f32, tag="sc")
                state = (kind, nt, mt, o_t, w_t, b_t, ps)
                proj_qk_mm(state, 0)
                proj_qk_mm(state, 1)
                proj_qk_copy(state)

            def proj_v_mm(st, vctx):
                # one 128-row chunk of sequence -> vsb[:, st, :, 0:64]
                nt, stl = st // 4, st % 4
                xc = xtiles[("v", nt)]
                ps = psS.tile([128, QT], f32, tag="sc")
                psv = ps[:, 0:DLOC]
                for c in range(KC):
                    nc.tensor.matmul(
                        psv,
                        xc[:, c, stl * 128:(stl + 1) * 128],
                        wv_t[:, c, :],
                        start=(c == 0), stop=(c == KC - 1),
                        skip_group_check=True)
                vctx[st] = psv

            def proj_v_copy(st, vctx):
                nc.scalar.copy(
                    vsb[:, st, :, 0:64],
                    vctx.pop(st).rearrange("p (h d) -> p h d", h=4))

            def proj_v_st(st):
                vctx = {}
                proj_v_mm(st, vctx)
                proj_v_copy(st, vctx)

            def out_mm(qt, mt, octx):
                qsl = slice(qt * QT, (qt + 1) * QT)
                py = psS.tile([128, QT], f32, tag="sc")
                for c in range(2):
                    nc.tensor.matmul(
                        py,
                        wo_t[:, c, mt * 128:(mt + 1) * 128],
                        aoT[:, c, qsl],
                        start=(c == 0), stop=(c == 1),
                        skip_group_check=True)
                octx[(qt, mt)] = py

            def out_copy(qt, mt, octx):
                py = octx.pop((qt, mt))
                ysb = ysbp.tile([128, QT], bf16, tag="y")
                if mt % 2 == 0:
                    nc.scalar.copy(ysb, py)
                else:
                    nc.vector.tensor_copy(ysb, py)
                nc.sync.dma_start(out=yT[mt, :, qt, :], in_=ysb)

            def out_proj_unit(qt, pt, j):
                octx = {}
                out_mm(qt, 2 * pt + j, octx)
                out_copy(qt, 2 * pt + j, octx)

            def norm_muls(av, rcp, hp, qsl):
                nc.vector.tensor_mul(
                    aoT[0:64, hp, qsl], av[0:64, 0, :], rcp[:, 0, :])
                nc.vector.tensor_mul(
                    aoT[64:128, hp, qsl], av[0:64, 1, :], rcp[:, 1, :])

            # --- interleave schedule: (qt, hp, kt) -> list of thunks.
            # Matmul thunks run at slot kt; the PSUM->SBUF copy runs at a
            # LATER kt, registered at the list head so it precedes that kt's
            # exp on ACT/DVE (its deps are then already satisfied: no
            # head-of-line blocking, and pool recycling stays acyclic). ---
            sched = {}

            def at(qt, hp, kt, fn, front=False):
                lst = sched.setdefault((qt, hp, kt), [])
                lst.insert(0, fn) if front else lst.append(fn)

            def sched_qk(qt, hp, kind, nt, mt, kts, copy_kt):
                o_t, w_t, b_t = ((kh, wk_t, bk_t) if kind == "k"
                                 else (qh, wq_t, bq_t))
                ctx = {}

                def mk_mm(half):
                    def fn():
                        if "s" not in ctx:
                            ps = psS.tile([128, QT], f32, tag="sc")
                            ctx["s"] = (kind, nt, mt, o_t, w_t, b_t, ps)
                        proj_qk_mm(ctx["s"], half)
                    return fn

                if len(kts) == 1:
                    def both():
                        mk_mm(0)()
                        mk_mm(1)()
                    at(qt, hp, kts[0], both)
                else:
                    at(qt, hp, kts[0], mk_mm(0))
                    at(qt, hp, kts[1], mk_mm(1))
                at(qt, hp, copy_kt, lambda: proj_qk_copy(ctx["s"]),
                   front=True)

            # qt0 hp0: projections per slot so each x tile's readers retire
            # early (frees xpool slots for the in-flight DMA queue); mt0 of
            # nt lands before scores kt=4nt, mt1 before hp1.
            for slot, (kind, nt, mt) in (
                    (0, ("k", 0, 1)), (1, ("q", 0, 1)), (2, ("k", 1, 0)),
                    (4, ("k", 1, 1)), (5, ("k", 2, 0)), (6, ("k", 2, 1)),
                    (8, ("k", 3, 0)), (10, ("k", 3, 1))):
                sched_qk(0, 0, kind, nt, mt, [slot], slot + 1)
            vctx = {}
            at(0, 0, 0, lambda: proj_v_mm(2, vctx))
            at(0, 0, 1, lambda: proj_v_mm(3, vctx))
            at(0, 0, 1, lambda: proj_v_copy(2, vctx), front=True)
            at(0, 0, 2, lambda: proj_v_copy(3, vctx), front=True)
            for st in range(4, 16):
                at(0, 0, st - 1, lambda s=st: proj_v_mm(s, vctx))
                at(0, 0, st, lambda s=st: proj_v_copy(s, vctx), front=True)
            # qt0 hp1: Q proj nt1
            sched_qk(0, 1, "q", 1, 0, [2, 3], 4)
            sched_qk(0, 1, "q", 1, 1, [9, 10], 11)
            # qt 1..2 hp0: Q proj qt+1
            for qt in range(1, NQT - 1):
                sched_qk(qt, 0, "q", qt + 1, 0, [6, 7], 8)
                sched_qk(qt, 0, "q", qt + 1, 1, [12, 13], 14)
            # qt >= 1 hp1: out projection of qt-1, mm at even kts,
            # cast+DMA at the following odd kt
            for qt in range(1, NQT):
                octx = {}
                for mt in range(8):
                    at(qt, 1, 2 * mt,
                       lambda q=qt - 1, m=mt, o=octx: out_mm(q, m, o))
                    at(qt, 1, 2 * mt + 1,
                       lambda q=qt - 1, m=mt, o=octx: out_copy(q, m, o),
                       front=True)

            # --- lead-in ---
            proj_qk_mt("k", 0, 0, kh, wk_t, bk_t)
            proj_qk_mt("q", 0, 0, qh, wq_t, bq_t)
            proj_v_st(0)
            proj_v_st(1)

            pending = None  # (av, hp, qsl) awaiting normalize
            ctx = {}

            # --- attention loop ---
            for qt in range(NQT):
                qsl = slice(qt * QT, (qt + 1) * QT)
                for hp in range(2):
                    av = psAV.tile([128, 2, QT], f32, tag="av")
                    esbs = {}
                    for kt in range(NKT):
                        ksl = slice(kt * 128, (kt + 1) * 128)
                        scA = psS.tile([128, QT], f32, tag="sc")
                        scB = psS.tile([128, QT], f32, tag="sc")
                        nc.tensor.matmul(
                            scA, kh[0:64, hp, ksl],
                            qh[0:64, hp, qsl], start=True, stop=True)
                        nc.tensor.matmul(
                            scB, kh[64:128, hp, ksl],
                            qh[64:128, hp, qsl], start=True, stop=True)
                        # interleaved work first: its PSUM->SBUF copies must
                        # precede this kt's exp in ACT/DVE program order, or
                        # the next scores' pool allocation deadlocks on them
                        for fn in sched.get((qt, hp, kt), ()):
                            fn()
                        # per-engine esb pools: each engine only ever rewrites
                        # its own buffers, so recycling needs no cross-engine
                        # write-after-write semaphores
                        dve_j = kt % 2
                        eD = esbDp.tile([128, QT], bf16, tag="eD")
                        eA = esbAp.tile([128, QT], bf16, tag="eA")
                        nc.vector.tensor_scalar(
                            eD[:].bitcast(i16), scA if dve_j == 0 else scB,
                            SCH_A, SCH_B, ALU.mult, ALU.add)
                        nc.scalar.activation(
                            eA, scB if dve_j == 0 else scA,
                            AF.Exp, scale=0.125)
                        esbs[kt] = (eA, eD, dve_j)
                        # software pipeline: AV of kt-1
                        if kt > 0:
                            eA1, eD1, dj1 = esbs.pop(kt - 1)
                            for j in range(2):
                                nc.tensor.matmul(
                                    av[:, j, :], vsb[:, kt - 1, 2 * hp + j, :],
                                    eD1 if j == dj1 else eA1,
                                    start=(kt - 1 == 0), stop=False,
                                    skip_group_check=True)
                        # deferred normalize chain for the previous segment:
                        # magic-constant reciprocal seed + one Newton step,
                        # all on DVE straight from the PSUM sums (no copy).
                        # high_priority pins the ops early in the scheduler's
                        # commit order so the chain can't drift past kt14.
                        if pending is not None:
                            psums = pending[0][64:128, :, :]
                            if kt == 0:
                                y0 = nrmp.tile([64, 2, QT], f32, tag="y0")
                                with tc.high_priority():
                                    nc.vector.tensor_scalar(
                                        y0[:].bitcast(i32),
                                        psums.bitcast(i32),
                                        -1, RECIP_MAGIC, ALU.mult, ALU.add)
                                ctx["y0"] = y0
                            elif kt == 1:
                                m = nrmp.tile([64, 2, QT], f32, tag="m")
                                with tc.high_priority():
                                    nc.vector.tensor_mul(
                                        m, psums, ctx["y0"])
                                ctx["m"] = m
                            elif kt == 2:
                                sn = nrmp.tile([64, 2, QT], f32, tag="s")
                                with tc.high_priority():
                                    nc.vector.tensor_scalar(
                                        sn, ctx["m"], -1.0, 2.0,
                                        ALU.mult, ALU.add)
                                ctx["s"] = sn
                            elif kt == 3:
                                rcp = nrmp.tile([64, 2, QT], f32, tag="rcp")
                                with tc.high_priority():
                                    nc.vector.tensor_mul(
                                        rcp, ctx["s"], ctx["y0"])
                                ctx["rcp"] = rcp
                            elif kt == 14:
                                with tc.high_priority():
                                    norm_muls(pending[0], ctx["rcp"],
                                              pending[1], pending[2])
                                pending = None
                    # AV of kt 15
                    eA1, eD1, dj1 = esbs.pop(NKT - 1)
                    for j in range(2):
                        nc.tensor.matmul(
                            av[:, j, :], vsb[:, NKT - 1, 2 * hp + j, :],
                            eD1 if j == dj1 else eA1,
                            start=False, stop=True,
                            skip_group_check=True)
                    pending = (av, hp, qsl)

            # tail: final normalize via exp(-ln); all 4 pairs start their
            # c=0 matmuls under the normalize latency
            qsl3 = slice((NQT - 1) * QT, NQT * QT)
            early = []
            for mt in range(4):
                py = psS.tile([128, QT], f32, tag="sc")
                nc.tensor.matmul(
                    py,
                    wo_t[:, 0, mt * 128:(mt + 1) * 128],
                    aoT[:, 0, qsl3],
                    start=True, stop=False, skip_group_check=True)
                early.append(py)
            psums = pending[0][64:128, :, :]
            y0 = nrmp.tile([64, 2, QT], f32, tag="y0")
            nc.vector.tensor_scalar(
                y0[:].bitcast(i32), psums.bitcast(i32),
                -1, RECIP_MAGIC, ALU.mult, ALU.add)
            m = nrmp.tile([64, 2, QT], f32, tag="m")
            nc.vector.tensor_mul(m, psums, y0)
            sn = nrmp.tile([64, 2, QT], f32, tag="s")
            nc.vector.tensor_scalar(sn, m, -1.0, 2.0, ALU.mult, ALU.add)
            rcp = nrmp.tile([64, 2, QT], f32, tag="rcp")
            nc.vector.tensor_mul(rcp, sn, y0)
            norm_muls(pending[0], rcp, pending[1], pending[2])
            for mt in range(4):
                py = early[mt]
                nc.tensor.matmul(
                    py,
                    wo_t[:, 1, mt * 128:(mt + 1) * 128],
                    aoT[:, 1, qsl3],
                    start=False, stop=True, skip_group_check=True)
                ysb = ysbp.tile([128, QT], bf16, tag="y")
                if mt % 2 == 0:
                    nc.scalar.copy(ysb, py)
                else:
                    nc.vector.tensor_copy(ysb, py)
                nc.sync.dma_start(out=yT[mt, :, NQT - 1, :], in_=ysb)
            for mt in range(4, 8):
                out_proj_unit(NQT - 1, mt // 2, mt % 2)
    nc.compile()
    return nc


def _get_nc():
    if "nc" not in _CACHE:
        _CACHE["nc"] = _build()
    return _CACHE["nc"]


def kernel(q, k, v, w_q, b_q, w_k, b_k, w_v, b_v, w_o, b_o, _trace=False):
    from concourse.bass_utils import run_bass_kernel_spmd

    bf = ml_dtypes.bfloat16
    q = np.asarray(q, np.float32)
    k = np.asarray(k, np.float32)
    v = np.asarray(v, np.float32)
    w_q = np.asarray(w_q, np.float32)
    w_k = np.asarray(w_k, np.float32)
    w_v = np.asarray(w_v, np.float32)
    w_o = np.asarray(w_o, np.float32)
    b_q = np.asarray(b_q, np.float32)
    b_k = np.asarray(b_k, np.float32)
    b_v = np.asarray(b_v, np.float32)
    b_o = np.asarray(b_o, np.float32)

    nc = _get_nc()

    def tile_x(x):
        # [S, D] -> [NQT, 128, KC, QT]: A[nt, p, c, s] = x[nt*QT+s, c*128+p]
        t = x.T.reshape(KC, 128, NQT, QT)
        return np.ascontiguousarray(t.transpose(2, 1, 0, 3)).astype(bf)

    def tile_w_qk(w, lo, hi):
        # [D, dloc] -> [128, 2, KC, 128]: W[p, mt, c, m] = w[lo+mt*128+m, c*128+p]
        t = w[lo:hi, :].T.reshape(KC, 128, 2, 128)
        return np.ascontiguousarray(t.transpose(1, 2, 0, 3)).astype(bf)

    def tile_w_v(w, lo, hi):
        # [D, dloc] -> [128, KC, dloc]
        t = w[lo:hi, :].T.reshape(KC, 128, DLOC)
        return np.ascontiguousarray(t.transpose(1, 0, 2)).astype(bf)

    xqT = [tile_x(q[b]) for b in range(B)]
    xkT = [tile_x(k[b]) for b in range(B)]
    xvT = [tile_x(v[b]) for b in range(B)]

    in_maps = []
    for c in range(N_CORES):
        b, hg = c // 4, c % 4
        lo, hi = hg * DLOC, (hg + 1) * DLOC
        in_maps.append({
            "xq": xqT[b],
            "xk": xkT[b],
            "xv": xvT[b],
            "wq": tile_w_qk(w_q, lo, hi),
            "wk": tile_w_qk(w_k, lo, hi),
            "wv": tile_w_v(w_v, lo, hi),
            "wo": np.ascontiguousarray(
                w_o[:, lo:hi].T.reshape(2, 128, D).transpose(1, 0, 2)
            ).astype(bf),
            "bq": np.ascontiguousarray(b_q[lo:hi].reshape(2, 128).T),
            "bk": np.ascontiguousarray(b_k[lo:hi].reshape(2, 128).T),
        })

    res = run_bass_kernel_spmd(
        nc, in_maps, core_ids=list(range(N_CORES)), trace=_trace)
    if _trace:
        _CACHE["last_result"] = res

    # b_v contributes exactly (w_o @ b_v) per output element (softmax rows
    # sum to 1); b_o adds directly.
    const_row = (b_o + w_o @ b_v).astype(np.float32)  # [D]
    out = np.empty((B, S, D), np.float32)
    for b in range(B):
        acc = res.results[4 * b]["yT"].astype(np.float32)
        for c in range(4 * b + 1, 4 * b + 4):
            acc += res.results[c]["yT"].astype(np.float32)
        out[b] = acc.reshape(D, S).T + const_row
    return out
